# revision 67
# baseline (speedup 1.0000x reference)
"""Fused attention + FC + residual + LayerNorm for Trainium2, 8 NeuronCores.

Problem: B=8, L=2048, d_k=d_v=64, d_model=1024, fp32 I/O.
Sharding: pure data parallel - batch element b -> core b. No collectives.

v25 design (engine-balanced pipeline, ScalarE exp is the pacing stream):

  LayerNorm scale-invariance: softmax normalization never applied.
  With et = C*exp(s) (C=1/32 folded into the ACT bias so et fits fp8e4),
  u = (et@V)@W^T unnormalized, D = sum_k et (ones-column in V):
      LN(u + D*(b + r)) == LN(softmax@V@W^T + b + r)
  eps must be rescaled: eps' = D^2 * eps (per-partition).

  - S^T: row-packed K=64 bf16 matmul pairs (tile_position halves).
  - exp on ScalarE, fp8e4 out; PV via a single fp8 DoubleRow matmul per
    k-tile pair (contraction 256): lhsT=[128,2,65] v+ones, rhs=[128,2,512].
  - FC row-packed bf16 (outU duplicated across partition halves).
  - Epilogue per 128-q tile: scalar_tensor_tensor x = (res*D) + fc_psum
    (bf16 out, accum_out = row-sum -> mean), tensor_tensor_reduce
    xsq = x*x (accum_out -> sumsq), var = E[x^2]-E[x]^2 + D^2 eps,
    rstd = exp(-0.5 ln(var)), apply = (x-mean)*rstd in bf16 (DVE 4x),
    store via SWDGE with bf16->f32 cast on the fly.
  - Full residual (8 MB) prefetched into SBUF at kernel start (4 x 2MB
    HWDGE chunks behind q/k/v/fcw); outputs stream out per-tile on the
    GPSIMD SWDGE ring.
"""
import numpy as np

B = 8
L = 2048
D = 64
DM = 1024
NTILES = L // 128       # 16 q/k tiles of 128
NSLICES = L // 512      # 4 q-slices of 512
LN_EPS = 1e-5
SCALE = 0.125           # 1/sqrt(64)
EXP_BIAS = float(np.log(1.0 / 32.0))  # C=1/32: keeps C*exp(s) < 448 (fp8e4 max)

_CACHE = {}
_TABLES_PATCHED = False


def _patch_act_tables():
    """Force every activation we use into one table set so the scheduler
    never needs a mid-kernel ACT_TABLE_LOAD switch (Exp <-> Ln)."""
    global _TABLES_PATCHED
    if _TABLES_PATCHED:
        return
    import concourse.bacc as bacc
    from concourse import mybir

    orig = bacc.get_activation_tables
    keep = "natural_log_exp_and_others"
    shared = {
        mybir.ActivationFunctionType.Exp,
        mybir.ActivationFunctionType.Ln,
        mybir.ActivationFunctionType.Copy,
        mybir.ActivationFunctionType.Identity,
        mybir.ActivationFunctionType.Square,
    }

    def patched(arch):
        tables = orig(arch)
        for name, fns in tables.items():
            if name != keep:
                fns.difference_update(shared)
        return tables

    bacc.get_activation_tables = patched
    _TABLES_PATCHED = True


def _build(affine: bool, with_bias: bool, use_fp8: bool = True):
    import concourse.bacc as bacc
    import concourse.tile as tile
    from concourse import mybir
    import concourse.bass as bass
    from concourse.masks import make_identity

    _patch_act_tables()
    f32 = mybir.dt.float32
    bf16 = mybir.dt.bfloat16
    fp8 = mybir.dt.float8e4
    et_dt = fp8 if use_fp8 else bf16
    ALU = mybir.AluOpType
    nc = bacc.Bacc("TRN2", target_bir_lowering=False, debug=False, num_devices=B)

    q_d = nc.declare_dram_parameter("q", [L, D], f32, isOutput=False)
    k_d = nc.declare_dram_parameter("k", [L, D], f32, isOutput=False)
    v_d = nc.declare_dram_parameter("v", [L, D], f32, isOutput=False)
    res_d = nc.declare_dram_parameter("residual", [L, DM], f32, isOutput=False)
    fcw_d = nc.declare_dram_parameter("fc_w", [DM, D], f32, isOutput=False)
    fcb_d = nc.declare_dram_parameter("fc_b", [DM], f32, isOutput=False)
    gam_d = nc.declare_dram_parameter("ln_gamma", [DM], f32, isOutput=False)
    bet_d = nc.declare_dram_parameter("ln_beta", [DM], f32, isOutput=False)
    out_d = nc.declare_dram_parameter("out", [L, DM], f32, isOutput=True)

    with tile.TileContext(nc) as tc:
        with (
            tc.tile_pool(name="raw", bufs=2) as raw_pool,
            tc.tile_pool(name="persist", bufs=1) as persist,
            tc.tile_pool(name="stage", bufs=2, space="PSUM") as stage_pool,
            tc.tile_pool(name="pv", bufs=1, space="PSUM") as pv_pool,
            tc.tile_pool(name="fc", bufs=3, space="PSUM") as fc_pool,
            tc.tile_pool(name="et", bufs=6) as et_pool,
            tc.tile_pool(name="x", bufs=8) as x_pool,
            tc.tile_pool(name="xsq", bufs=2) as xsq_pool,
            tc.tile_pool(name="outs", bufs=8) as out_pool,
            tc.tile_pool(name="norm", bufs=2) as norm_pool,
            tc.tile_pool(name="small", bufs=4) as small_pool,
        ):
            identity = persist.tile([128, 128], f32)
            make_identity(nc, identity)
            one_c = persist.tile([128, 1], bf16, tag="onec")
            nc.vector.memset(one_c, 1.0)
            ebias = persist.tile([128, 1], f32, tag="ebias")
            nc.vector.memset(ebias, EXP_BIAS)

            # ---- split loads: first halves gate attention(0); k first ----
            vraw = raw_pool.tile([128, NTILES, D], f32, tag="vraw")
            v_view = v_d.ap().rearrange("(p t) d -> p t d", p=128)
            nc.gpsimd.dma_start(out=vraw[:, 0:8, :], in_=v_view[:, 0:8, :])
            qT2 = persist.tile([128, NTILES, 128], bf16, tag="qT")
            kT2 = persist.tile([128, NTILES, 128], bf16, tag="kT")
            kraw = raw_pool.tile([128, NTILES, D], f32, tag="kraw")
            qraw = raw_pool.tile([128, NTILES, D], f32, tag="qraw")
            k_view = k_d.ap().rearrange("(p t) d -> p t d", p=128)
            q_view = q_d.ap().rearrange("(p t) d -> p t d", p=128)
            nc.scalar.dma_start(out=kraw[:, 0:8, :], in_=k_view[:, 0:8, :])
            nc.sync.dma_start(out=kraw[:, 8:16, :], in_=k_view[:, 8:16, :])
            nc.sync.dma_start(out=qraw[:, 0:8, :], in_=q_view[:, 0:8, :])
            nc.gpsimd.dma_start(out=vraw[:, 8:16, :], in_=v_view[:, 8:16, :])
            nc.scalar.dma_start(out=qraw[:, 8:16, :], in_=q_view[:, 8:16, :])

            # fc_w raw ahead of the big residual chunks on the sync ring
            fraw = raw_pool.tile([128, DM // 128, D], f32, tag="fraw")
            nc.sync.dma_start(
                out=fraw,
                in_=fcw_d.ap().rearrange("(t p) d -> p t d", p=128),
            )

            # ---- full-residual prefetch: 4 x 2MB chunks on sync HWDGE ----
            res_sb = persist.tile([128, NTILES, DM], f32, tag="res")
            res_view = res_d.ap().rearrange("(p t) m -> p t m", p=128)
            out_view = out_d.ap().rearrange("(p t) m -> p t m", p=128)
            for c in range(NSLICES):
                nc.sync.dma_start(
                    out=res_sb[:, 4 * c:4 * c + 4, :],
                    in_=res_view[:, 4 * c:4 * c + 4, :],
                )

            def transpose_group(raw, dstT, grp, ring):
                dlo = dstT[0:64, :, :].rearrange(
                    "d (g pair par) c -> d g pair par c", pair=4, par=2)
                pt = stage_pool.tile([128, 512], f32, tag="stage")
                for i in range(4):
                    nc.tensor.transpose(
                        pt[:, i * 128:(i + 1) * 128],
                        raw[:, (8 * grp + 2 * i): (8 * grp + 2 * i + 2), :],
                        identity,
                    )
                ptv = pt.rearrange("p (four c) -> p four c", c=128)
                nc.vector.tensor_copy(dlo[:, grp, :, 0, :], ptv[0:64])
                nc.vector.tensor_copy(dlo[:, grp, :, 1, :], ptv[64:128])
                nc.vector.tensor_copy(
                    dstT[64:128, 8 * grp:8 * grp + 8, :],
                    dstT[0:64, 8 * grp:8 * grp + 8, :],
                )

            # k fully (attention streams over all k-tiles), q slice-0 half
            transpose_group(kraw, kT2, 0, nc.scalar)
            transpose_group(kraw, kT2, 1, nc.scalar)
            transpose_group(qraw, qT2, 0, nc.sync)

            # ---- v with ones column (fp8: strided pair layout for DR) ----
            VPAD = 80 if use_fp8 else D + 1
            v_sb = persist.tile([128, NTILES, VPAD], et_dt, tag="v")
            nc.scalar.copy(v_sb[:, 0:8, 0:D], vraw[:, 0:8, :])
            nc.scalar.copy(v_sb[:, 8:16, 0:D], vraw[:, 8:16, :])
            nc.vector.memset(v_sb[:, :, D:D + 1], 1.0)

            fcwT = persist.tile([128, DM], bf16, tag="fcw")

            def fcw_prep():
                # FC runs unpacked (K=64) so only rows 0:64 are needed
                flo = fcwT[0:64, :].rearrange(
                    "d (pair par c) -> d pair par c", par=2, c=128)
                pt = stage_pool.tile([128, 512], f32, tag="stage")
                for i in range(4):
                    nc.tensor.transpose(
                        pt[:, i * 128:(i + 1) * 128],
                        fraw[:, 2 * i: 2 * i + 2, :],
                        identity,
                    )
                ptv = pt.rearrange("p (four c) -> p four c", c=128)
                nc.vector.tensor_copy(flo[:, :, 0, :], ptv[0:64])
                nc.vector.tensor_copy(flo[:, :, 1, :], ptv[64:128])

            if with_bias:
                fcb_bc = persist.tile([128, DM], f32, tag="fcb")
                nc.sync.dma_start(
                    out=fcb_bc,
                    in_=bass.AP(tensor=fcb_d, offset=0, ap=[[0, 128], [1, DM]]),
                )
                # fold the fc bias into the residual once up front
                for t in range(NTILES):
                    nc.vector.tensor_add(res_sb[:, t, :], res_sb[:, t, :],
                                         fcb_bc)
            if affine:
                gam_bc = persist.tile([128, DM], f32, tag="gam")
                bet_bc = persist.tile([128, DM], f32, tag="bet")
                nc.sync.dma_start(
                    out=gam_bc,
                    in_=bass.AP(tensor=gam_d, offset=0, ap=[[0, 128], [1, DM]]),
                )
                nc.sync.dma_start(
                    out=bet_bc,
                    in_=bass.AP(tensor=bet_d, offset=0, ap=[[0, 128], [1, DM]]),
                )

            state = {}
            ngrp = NTILES // 2

            def make_attention(s):
                qlo = qT2[0:64, :, :].rearrange("d t c -> d (t c)")[
                    :, s * 512:(s + 1) * 512]
                qhi = qT2[64:128, :, :].rearrange("d t c -> d (t c)")[
                    :, s * 512:(s + 1) * 512]
                out_aug = pv_pool.tile([65, 512], f32, tag="pv")

                def s_pair(g):
                    # row-packed: k-tile 2g in rows 0:63, 2g+1 in 64:127
                    st = stage_pool.tile([128, 1024], f32, tag="stage")
                    nc.tensor.matmul(st[:, 0:512], kT2[0:64, 2 * g, :], qlo,
                                     start=True, stop=True,
                                     tile_position=(0, 0))
                    nc.tensor.matmul(st[:, 512:1024],
                                     kT2[64:128, 2 * g + 1, :],
                                     qhi, start=True, stop=True,
                                     tile_position=(64, 0))
                    return st

                def exp_pv(g, st):
                    et = et_pool.tile([128, 1024], et_dt, tag="et")
                    nc.scalar.activation(
                        out=et, in_=st,
                        func=mybir.ActivationFunctionType.Exp, scale=SCALE,
                        bias=ebias,
                    )
                    if use_fp8:
                        et_v = et.rearrange("p (two n) -> p two n", two=2)
                        nc.tensor.matmul(
                            out_aug, v_sb[:, 2 * g:2 * g + 2, 0:D + 1], et_v,
                            start=(g == 0), stop=(g == ngrp - 1),
                            perf_mode=mybir.MatmulPerfMode.DoubleRow,
                        )
                    else:
                        nc.tensor.matmul(out_aug, v_sb[:, 2 * g, 0:D + 1],
                                         et[:, 0:512],
                                         start=(g == 0), stop=False)
                        nc.tensor.matmul(out_aug, v_sb[:, 2 * g + 1, 0:D + 1],
                                         et[:, 512:1024],
                                         start=False, stop=(g == ngrp - 1))

                return out_aug, s_pair, exp_pv

            def attention_pre(s):
                # the opening S^T pairs: hoisted before the previous
                # slice's final exp so exp(s,0) can follow exp(s-1,7)
                # back-to-back on ScalarE (kills the boundary stall)
                out_aug, s_pair, exp_pv = make_attention(s)
                st0 = s_pair(0)
                st1 = s_pair(1)
                return [out_aug, s_pair, exp_pv, st0, st1]

            def attention_p1(pr):
                out_aug, s_pair, exp_pv, st0, st1 = pr
                exp_pv(0, st0)
                return out_aug, [s_pair, exp_pv, st1]

            def drain(work, n):
                for _ in range(n):
                    if work:
                        work.pop(0)()

            def attention_p2a(ctx, work):
                s_pair, exp_pv, st_prev = ctx
                for g in range(2, 6):
                    st_cur = s_pair(g)
                    exp_pv(g - 1, st_prev)
                    # spread the previous slice's FC/STT chain through the
                    # pair loop so it never head-blocks the PE FIFO
                    drain(work, 2)
                    st_prev = st_cur
                ctx[2] = st_prev

            def attention_p2b(ctx, work, before_last=None):
                s_pair, exp_pv, st_prev = ctx
                for g in range(6, ngrp):
                    st_cur = s_pair(g)
                    exp_pv(g - 1, st_prev)
                    drain(work, 2)
                    st_prev = st_cur
                if before_last is not None:
                    # bn-tile stats/applies overlap the final pair instead
                    # of spilling onto the slice boundary
                    before_last()
                exp_pv(ngrp - 1, st_prev)
                drain(work, len(work))

            def dance(s, out_aug):
                # single evac of PV result incl. denominator row 64 (bf16)
                outU = norm_pool.tile([65, 512], bf16, tag="outU")
                nc.vector.tensor_copy(outU, out_aug)
                dT = small_pool.tile([128, 4], f32, tag="dT")
                epsT = small_pool.tile([128, 4], f32, tag="epsT")
                state[s] = {"outU": outU, "dT": dT, "epsT": epsT}

            def dance_tail(s):
                # deferred D-transpose: drained as the first work item so
                # the dps matmuls sit BEHIND sp(s+1,2) in the PE FIFO
                outU = state[s]["outU"]
                dT = state[s]["dT"]
                epsT = state[s]["epsT"]
                dps = stage_pool.tile([128, 4], f32, tag="stage")
                for t in range(4):
                    nc.tensor.matmul(dps[:, t:t + 1],
                                     outU[64:65, t * 128:(t + 1) * 128],
                                     one_c[64:65, :], start=True, stop=True,
                                     tile_position=(64, 0))
                # tensor_scalar sidesteps the ~685ns DVE copy floor; dT
                # gates the first STT of the epilogue
                nc.vector.tensor_scalar_mul(out=dT, in0=dps, scalar1=1.0)
                nc.vector.tensor_mul(epsT, dT, dT)
                nc.vector.tensor_scalar_mul(out=epsT, in0=epsT,
                                            scalar1=LN_EPS)

            def epilogue_a(s):
                """Build the per-tile FC/STT/stats emission thunks.

                Tiles 2,3 (emitted first) get their variance via ScalarE
                Square+accum (x ready early, Square slots between exps);
                tiles 0,1 use DVE bn_stats. Returns the thunk list that
                attention_p2* drains between pair groups."""
                outU = state[s]["outU"]
                dT = state[s]["dT"]
                mv_all = small_pool.tile([128, 2, 2], f32, tag="mv")
                sumx = small_pool.tile([128, 4], f32, tag="sumx")
                sumsq = small_pool.tile([128, 4], f32, tag="sumsq")
                x_ts = {}
                work = []

                def fc_half(pi, h):
                    fc_ps = fc_pool.tile([128, 512], f32, tag="fc")
                    nc.tensor.matmul(fc_ps,
                                     outU[0:64, pi * 128:(pi + 1) * 128],
                                     fcwT[0:64, h * 512:(h + 1) * 512],
                                     start=True, stop=True)
                    t = s * 4 + pi
                    x_t = x_ts[pi]
                    nc.vector.scalar_tensor_tensor(
                        out=x_t[:, h * 512:(h + 1) * 512],
                        in0=res_sb[:, t, h * 512:(h + 1) * 512],
                        scalar=dT[:, pi:pi + 1],
                        in1=fc_ps, op0=ALU.mult, op1=ALU.add,
                        accum_out=sumx[:, 2 * (pi - 2) + h:2 * (pi - 2) + h + 1]
                        if pi >= 2 else None,
                    )
                    return x_t

                def make_thunk(pi, h, bn_slot):
                    def thunk():
                        if pi not in x_ts:
                            x_ts[pi] = x_pool.tile([128, DM], f32, tag="x",
                                                   name="x_t")
                        x_t = fc_half(pi, h)
                        if bn_slot is not None:
                            stats = bn_slot
                            nc.vector.bn_stats(
                                out=stats[:, h, :],
                                in_=x_t[:, h * 512:(h + 1) * 512])
                            if h == 1:
                                nc.vector.bn_aggr(out=mv_all[:, pi, :],
                                                  in_=stats)
                        elif h == 1:
                            xsq = xsq_pool.tile([128, DM], bf16, tag="xsq")
                            nc.scalar.activation(
                                out=xsq, in_=x_t,
                                func=mybir.ActivationFunctionType.Square,
                                accum_out=sumsq[:, pi:pi + 1],
                            )
                    return thunk

                def stats_sq_thunk():
                    # mean/var/rstd for the Square tiles (2,3); emitted
                    # mid-work so the DVE smalls sit early in its FIFO
                    epsT = state[s]["epsT"]
                    meanA = small_pool.tile([128, 2], f32, tag="meanA")
                    sx = sumx.rearrange("p (t h) -> p t h", h=2)
                    nc.vector.tensor_add(meanA, sx[:, :, 0], sx[:, :, 1])
                    nc.vector.tensor_scalar_mul(out=meanA, in0=meanA,
                                                scalar1=1.0 / DM)
                    m2 = small_pool.tile([128, 2], f32, tag="m2")
                    nc.vector.tensor_mul(m2, meanA, meanA)
                    varA = small_pool.tile([128, 2], f32, tag="varA")
                    nc.vector.scalar_tensor_tensor(
                        out=varA, in0=sumsq[:, 2:4], scalar=1.0 / DM,
                        in1=m2, op0=ALU.mult, op1=ALU.subtract,
                    )
                    nc.vector.tensor_add(varA, varA, epsT[:, 2:4])
                    rstdA = small_pool.tile([128, 2], f32, tag="rstdA")
                    nc.scalar.activation(
                        out=rstdA, in_=varA,
                        func=mybir.ActivationFunctionType.Ln)
                    nc.scalar.activation(
                        out=rstdA, in_=rstdA,
                        func=mybir.ActivationFunctionType.Exp, scale=-0.5)
                    state[s]["meanA"] = meanA
                    state[s]["rstdA"] = rstdA

                # square-tiles (2,3) first: their x is ready early so the
                # ScalarE Squares never stall the exp stream
                work.append(lambda: dance_tail(s))
                for pi in (2, 3):
                    for h in (0, 1):
                        work.append(make_thunk(pi, h, None))
                work.append(stats_sq_thunk)
                for pi in (0, 1):
                    stats = small_pool.tile([128, 2, 6], f32,
                                            tag=f"stats{pi}")
                    for h in (0, 1):
                        work.append(make_thunk(pi, h, stats))
                state[s]["x_ts"] = x_ts
                state[s]["mv"] = mv_all
                state[s]["sumx"] = sumx
                state[s]["sumsq"] = sumsq
                return work

            def apply_tiles(s, pis, mean_t, rstd_t):
                x_ts = state[s]["x_ts"]
                for j, pi in enumerate(pis):
                    t = s * 4 + pi
                    out_t = out_pool.tile([128, DM], f32, tag="out")
                    nc.vector.tensor_scalar(
                        out=out_t, in0=x_ts[pi],
                        scalar1=mean_t[:, j:j + 1],
                        scalar2=rstd_t[:, j:j + 1],
                        op0=ALU.subtract,
                        op1=ALU.mult,
                    )
                    if affine:
                        nc.vector.tensor_mul(out_t, out_t, gam_bc)
                        nc.vector.tensor_add(out_t, out_t, bet_bc)
                    # SWDGE store (gpsimd): keeps the issue cost off the
                    # busy ScalarE/sync sequencers
                    nc.gpsimd.dma_start(
                        out=out_view[:, t, :], in_=out_t
                    )

            def epilogue_b_sq(s):
                apply_tiles(s, (2, 3), state[s]["meanA"], state[s]["rstdA"])

            def epilogue_b_bn(s):
                epsT = state[s]["epsT"]
                mv_all = state[s]["mv"]
                meanB = small_pool.tile([128, 2], f32, tag="meanB")
                nc.vector.tensor_copy(meanB, mv_all[:, :, 0])
                varB = small_pool.tile([128, 2], f32, tag="varB")
                nc.vector.tensor_add(varB, mv_all[:, :, 1], epsT[:, 0:2])
                rstdB = small_pool.tile([128, 2], f32, tag="rstdB")
                nc.scalar.activation(
                    out=rstdB, in_=varB,
                    func=mybir.ActivationFunctionType.Ln)
                nc.scalar.activation(
                    out=rstdB, in_=rstdB,
                    func=mybir.ActivationFunctionType.Exp, scale=-0.5)
                apply_tiles(s, (0, 1), meanB, rstdB)
                del state[s]

            def epilogue_last(s):
                """Tail epilogue: fully per-tile pipelined so the first
                store issues ~3 tiles earlier than the batched path."""
                dance_tail(s)
                outU = state[s]["outU"]
                dT = state[s]["dT"]
                epsT = state[s]["epsT"]
                sumx = small_pool.tile([128, 8], f32, tag="sumxL")
                sumsq = small_pool.tile([128, 4], f32, tag="sumsqL")
                meanL = small_pool.tile([128, 4], f32, tag="meanL")
                varL = small_pool.tile([128, 4], f32, tag="varL")
                rstdL = small_pool.tile([128, 4], f32, tag="rstdL")
                nmL = small_pool.tile([128, 4], f32, tag="nmL")
                sx = sumx.rearrange("p (t h) -> p t h", h=2)
                for pi in range(4):
                    t = s * 4 + pi
                    x_t = x_pool.tile([128, DM], f32, tag="x", name="x_t")
                    for h in (0, 1):
                        fc_ps = fc_pool.tile([128, 512], f32, tag="fc")
                        nc.tensor.matmul(
                            fc_ps, outU[0:64, pi * 128:(pi + 1) * 128],
                            fcwT[0:64, h * 512:(h + 1) * 512],
                            start=True, stop=True)
                        nc.vector.scalar_tensor_tensor(
                            out=x_t[:, h * 512:(h + 1) * 512],
                            in0=res_sb[:, t, h * 512:(h + 1) * 512],
                            scalar=dT[:, pi:pi + 1],
                            in1=fc_ps, op0=ALU.mult, op1=ALU.add,
                            accum_out=sumx[:, 2 * pi + h:2 * pi + h + 1])
                    xsq = xsq_pool.tile([128, DM], bf16, tag="xsq")
                    nc.scalar.activation(
                        out=xsq, in_=x_t,
                        func=mybir.ActivationFunctionType.Square,
                        accum_out=sumsq[:, pi:pi + 1])
                    mean1 = meanL[:, pi:pi + 1]
                    nc.vector.tensor_add(mean1, sx[:, pi, 0:1],
                                         sx[:, pi, 1:2])
                    nc.vector.tensor_scalar_mul(out=mean1, in0=mean1,
                                                scalar1=1.0 / DM)
                    v1 = varL[:, pi:pi + 1]
                    nc.vector.tensor_mul(v1, mean1, mean1)
                    nc.vector.scalar_tensor_tensor(
                        out=v1, in0=sumsq[:, pi:pi + 1], scalar=1.0 / DM,
                        in1=v1, op0=ALU.mult, op1=ALU.subtract)
                    nc.vector.tensor_add(v1, v1, epsT[:, pi:pi + 1])
                    r1 = rstdL[:, pi:pi + 1]
                    nc.scalar.activation(
                        out=r1, in_=v1,
                        func=mybir.ActivationFunctionType.Ln)
                    nc.scalar.activation(
                        out=r1, in_=r1,
                        func=mybir.ActivationFunctionType.Exp, scale=-0.5)
                    out_t = out_pool.tile([128, DM], f32, tag="out")
                    if pi % 2 == 1:
                        # odd tiles: apply on ScalarE (idle after the exps)
                        nm1 = nmL[:, pi:pi + 1]
                        nc.vector.tensor_mul(nm1, mean1, r1)
                        nc.vector.tensor_scalar_mul(out=nm1, in0=nm1,
                                                    scalar1=-1.0)
                        nc.scalar.activation(
                            out=out_t, in_=x_t,
                            func=mybir.ActivationFunctionType.Identity,
                            bias=nm1, scale=r1)
                    else:
                        nc.vector.tensor_scalar(
                            out=out_t, in0=x_t, scalar1=mean1, scalar2=r1,
                            op0=ALU.subtract, op1=ALU.mult)
                    if affine:
                        nc.vector.tensor_mul(out_t, out_t, gam_bc)
                        nc.vector.tensor_add(out_t, out_t, bet_bc)
                    nc.gpsimd.dma_start(
                        out=out_view[:, t, :], in_=out_t)
                del state[s]

            # pipeline: previous slice's dance + epilogue thunks drain
            # between the next attention's pair groups; the Square tiles'
            # LN applies go mid-slice, the bn tiles' at the slice end
            oa = {}
            actx = None
            pre = {}
            for s in range(NSLICES + 1):
                if s < NSLICES:
                    if s not in pre:
                        pre[s] = attention_pre(s)
                    oa[s], actx = attention_p1(pre.pop(s))
                work = []
                if s - 1 >= 0:
                    dance(s - 1, oa.pop(s - 1))
                    if s == NSLICES:
                        epilogue_last(s - 1)
                    else:
                        work = epilogue_a(s - 1)
                if s < NSLICES:
                    attention_p2a(actx, work)
                    epilogue_b_sq(s - 1) if s - 1 >= 0 else None
                    sp = s - 1
                    sn = s + 1

                    def before_last(sp=sp, sn=sn):
                        if sp >= 0:
                            epilogue_b_bn(sp)
                        if sn < NSLICES:
                            pre[sn] = attention_pre(sn)

                    attention_p2b(actx, work, before_last=before_last)
                if s == 0:
                    # deferred: not needed until slice 1 / epilogue(0)
                    transpose_group(qraw, qT2, 1, nc.sync)
                    fcw_prep()

    nc.finalize()
    return nc


LAST_RESULTS = None


def kernel(q, k, v, residual, fc_w, fc_b, ln_gamma, ln_beta):
    from concourse.bass_utils import run_bass_kernel_spmd

    global LAST_RESULTS
    affine = not (
        np.allclose(ln_gamma, 1.0) and np.allclose(ln_beta, 0.0)
    )
    with_bias = not np.all(np.asarray(fc_b) == 0.0)
    key = ("v25b", affine, with_bias)
    if key not in _CACHE:
        _CACHE[key] = _build(affine, with_bias)
    nc = _CACHE[key]

    q = np.ascontiguousarray(q, dtype=np.float32)
    k = np.ascontiguousarray(k, dtype=np.float32)
    v = np.ascontiguousarray(v, dtype=np.float32)
    residual = np.ascontiguousarray(residual, dtype=np.float32)
    fc_w = np.ascontiguousarray(fc_w, dtype=np.float32)
    fc_b = np.ascontiguousarray(fc_b, dtype=np.float32)
    ln_gamma = np.ascontiguousarray(ln_gamma, dtype=np.float32)
    ln_beta = np.ascontiguousarray(ln_beta, dtype=np.float32)

    in_maps = [
        {
            "q": q[b], "k": k[b], "v": v[b], "residual": residual[b],
            "fc_w": fc_w, "fc_b": fc_b,
            "ln_gamma": ln_gamma, "ln_beta": ln_beta,
        }
        for b in range(B)
    ]
    res = run_bass_kernel_spmd(nc, in_maps, core_ids=list(range(B)))
    LAST_RESULTS = res
    return np.stack([res.results[b]["out"] for b in range(B)], axis=0)
